# revision 10
# baseline (speedup 1.0000x reference)
"""Trainium2 Bass kernel for CausalSelfAttention variant (B=4, N=2048, D=1024, H=16, dk=dv=64).

Reference quirks faithfully implemented:
  - softmax over axis=2 (query axis): A[:, j] normalized over i (column softmax)
  - no 1/sqrt(dk) scaling
  - raw reshape (B,H,N,dv) -> (B,N,H*dv): output rows g*128:(g+1)*128 of batch b
    depend only on head g: out[b, g*128+r, :] = AV[b,g].reshape(128,1024)[r] @ W_O

Sharding (8 cores): core c handles batch b=c//2, global heads (c%2)*8 .. +8.
Each core produces out[b, (c%2)*1024 : +1024, :].

Precision scheme (target ~3e-4 vs fp32 reference): every fp32 value feeding the
softmax exponent goes through an fp16 hi+lo split (hi = fp16(x), lo = fp16(x-hi))
and products are computed as hi*hi + hi*lo + lo*hi (3-pass, error ~2^-21).
That covers X^T, W_Q/W_K, the Q/K projections and S^T itself. exp uses a
constant -14 bias so E=exp(S-14) and Vtil=V*(exp(14)/c) both fit fp16; the AV
matmul then runs single-pass fp16 (rounding averages out across the softmax
support). The O projection re-accumulates the exact AV rearrangement in PSUM
and runs 3-pass fp16 against split W_O.
"""

import numpy as np
from contextlib import ExitStack

import concourse.bass as bass
import concourse.tile as tile
from concourse import bacc, mybir
from concourse.bass_utils import run_bass_kernel_spmd

B, N, D, H, DK, DV = 4, 2048, 1024, 16, 64, 64
NCORES = 8
HPC = 8          # heads per core
F32 = mybir.dt.float32
F16 = mybir.dt.float16
EXP = mybir.ActivationFunctionType.Exp
EXP_BIAS = -14.0

_compiled = None


def _identity_np():
    # cols 0:256 : ID[p, 192*par + p%64] = 1  (avt -> avt2 rearrangement lhsT)
    # cols 256:384: plain I128 (PE-transpose identity)
    ident = np.zeros((128, 384), np.float32)
    for p in range(128):
        for par in range(2):
            ident[p, 192 * par + (p % 64)] = 1.0
        ident[p, 256 + p] = 1.0
    return ident


def _build():
    nc = bacc.Bacc("TRN2", target_bir_lowering=False, debug=False,
                   num_devices=NCORES)
    x_d = nc.dram_tensor("X", [N, D], F32, kind="ExternalInput").ap()
    wq_d = nc.dram_tensor("WQ", [HPC, D, DK], F32, kind="ExternalInput").ap()
    wk_d = nc.dram_tensor("WK", [HPC, D, DK], F32, kind="ExternalInput").ap()
    wv_d = nc.dram_tensor("WV", [HPC, D, DV], F32, kind="ExternalInput").ap()
    wo_d = nc.dram_tensor("WO", [D, D], F32, kind="ExternalInput").ap()
    id_d = nc.dram_tensor("ID", [128, 384], F32, kind="ExternalInput").ap()
    out_d = nc.dram_tensor("OUT", [HPC * 128, D], F32, kind="ExternalOutput").ap()

    with tile.TileContext(nc) as tc:
        with ExitStack() as ctx:
            persist = ctx.enter_context(tc.tile_pool(name="persist", bufs=1))
            stage = ctx.enter_context(tc.tile_pool(name="stage", bufs=2))
            wpool = ctx.enter_context(tc.tile_pool(name="w", bufs=1))
            vpool = ctx.enter_context(tc.tile_pool(name="v", bufs=1))
            qkpool = ctx.enter_context(tc.tile_pool(name="qk", bufs=2))
            epool = ctx.enter_context(tc.tile_pool(name="e", bufs=3))
            avtpool = ctx.enter_context(tc.tile_pool(name="avt", bufs=1))
            a2pool = ctx.enter_context(tc.tile_pool(name="a2", bufs=2))
            opool = ctx.enter_context(tc.tile_pool(name="o", bufs=2))
            small = ctx.enter_context(tc.tile_pool(name="sm", bufs=8))
            zpool = ctx.enter_context(tc.tile_pool(name="z", bufs=1))
            psp = ctx.enter_context(tc.tile_pool(name="ps", bufs=2, space="PSUM"))
            avp = ctx.enter_context(tc.tile_pool(name="avp", bufs=1, space="PSUM"))

            # ---- one-time init ----
            xhi = persist.tile([128, 8, N], F16, tag="xhi")      # X^T hi [d, i]
            xlo = persist.tile([128, 8, N], F16, tag="xlo")      # X^T lo
            wohi = persist.tile([128, 8, D], F16, tag="wohi")    # WO[128k+p, :]
            wolo = persist.tile([128, 8, D], F16, tag="wolo")
            idr = persist.tile([128, 384], F16, tag="idr")

            st = stage.tile([128, 1024], F32, tag="stg")
            nc.sync.dma_start(st[:, 0:384], id_d[:])
            nc.vector.tensor_copy(idr[:], st[:, 0:384])
            idT = idr[:, 256:384]

            for k in range(8):
                st = stage.tile([128, 1024], F32, tag="stg")
                nc.sync.dma_start(st[:], wo_d[k * 128:(k + 1) * 128, :])
                nc.vector.tensor_copy(wohi[:, k, :], st[:])
                nc.vector.tensor_sub(wolo[:, k, :], st[:], wohi[:, k, :])

            # X -> fp16 hi/lo -> PE-transpose into xhi/xlo
            for it in range(16):
                st = stage.tile([128, 1024], F32, tag="stg")
                nc.sync.dma_start(st[:], x_d[it * 128:(it + 1) * 128, :])
                sthi = stage.tile([128, D], F16, tag="sthi")
                stlo = stage.tile([128, D], F16, tag="stlo")
                nc.vector.tensor_copy(sthi[:], st[:])
                nc.vector.tensor_sub(stlo[:], st[:], sthi[:])
                for td in range(8):
                    ph = psp.tile([128, 128], F16, tag="ps")
                    nc.tensor.transpose(ph[:], sthi[:, td * 128:(td + 1) * 128],
                                        idT)
                    nc.vector.tensor_copy(
                        xhi[:, td, it * 128:(it + 1) * 128], ph[:])
                    pl = psp.tile([128, 128], F16, tag="ps")
                    nc.tensor.transpose(pl[:], stlo[:, td * 128:(td + 1) * 128],
                                        idT)
                    nc.scalar.copy(
                        xlo[:, td, it * 128:(it + 1) * 128], pl[:])

            z32 = zpool.tile([128, 128], F32, tag="z32")
            nc.vector.memset(z32[:], 0.0)
            z512 = zpool.tile([1, 512], F32, tag="z512")
            nc.vector.memset(z512[:], 0.0)
            ebias = zpool.tile([128, 1], F32, tag="ebias")
            nc.vector.memset(ebias[:], EXP_BIAS)
            zl = zpool.tile([1, 128], F16, tag="zl")
            zr = zpool.tile([1, 512], F16, tag="zr")
            nc.vector.tensor_copy(zl[:], z32[0:1, :])
            nc.vector.tensor_copy(zr[:], z512[:])
            # persistent [128, 128] Vtil tiles; the unused half stays zero so a
            # full-M matmul adds zero rows to the other head's AV partitions
            vt2 = []
            for hh in range(2):
                t = zpool.tile([128, 128], F16, tag=f"vt2_{hh}")
                nc.vector.tensor_copy(t[:], z32[:])
                vt2.append(t)

            for p2 in range(2):             # groups of 4 heads
                # V projection for heads 4*p2 .. 4*p2+3, packed N=256
                wv4 = wpool.tile([128, 8, 256], F16, tag="wv4")
                for h4 in range(4):
                    g = 4 * p2 + h4
                    st = stage.tile([128, 1024], F32, tag="stg")
                    stv = st[:, 0:512].rearrange("p (t k) -> p t k", k=64)
                    nc.sync.dma_start(
                        stv, wv_d[g].rearrange("(t p) k -> p t k", p=128))
                    nc.vector.tensor_copy(wv4[:, :, h4 * 64:(h4 + 1) * 64], stv)
                v4 = vpool.tile([128, 16, 256], F16, tag="v4")
                for jt in range(16):
                    psv = psp.tile([128, 1024], F32, tag="ps")
                    for d in range(8):
                        nc.tensor.matmul(psv[:, 0:256],
                                         xhi[:, d, jt * 128:(jt + 1) * 128],
                                         wv4[:, d, :],
                                         start=(d == 0), stop=(d == 7))
                    nc.vector.tensor_copy(v4[:, jt, :], psv[:, 0:256])

                for pp in range(2):         # head pairs within the group
                    pair = 2 * p2 + pp
                    # -- load packed W for the pair, split hi/lo --
                    wqh = wpool.tile([128, 8, 128], F16, tag="wqh")
                    wql = wpool.tile([128, 8, 128], F16, tag="wql")
                    wkh = wpool.tile([128, 8, 128], F16, tag="wkh")
                    wkl = wpool.tile([128, 8, 128], F16, tag="wkl")
                    for (wh, wl), wsrc in (((wqh, wql), wq_d),
                                           ((wkh, wkl), wk_d)):
                        st = stage.tile([128, 1024], F32, tag="stg")
                        stw = st[:].rearrange("p (t k) -> p t k", k=128)
                        for hh in range(2):
                            g = 2 * pair + hh
                            nc.sync.dma_start(
                                stw[:, :, hh * 64:(hh + 1) * 64],
                                wsrc[g].rearrange("(t p) k -> p t k", p=128))
                        nc.vector.tensor_copy(wh[:], stw)
                        nc.vector.tensor_sub(wl[:].rearrange("p t k -> p (t k)"),
                                             st[:],
                                             wh[:].rearrange("p t k -> p (t k)"))

                    # -- QK projections (3-pass split): QT/KT hi+lo packed --
                    qth = qkpool.tile([128, N], F16, tag="qth")
                    qtl = qkpool.tile([128, N], F16, tag="qtl")
                    kth = qkpool.tile([128, N], F16, tag="kth")
                    ktl = qkpool.tile([128, N], F16, tag="ktl")
                    for (dh, dl), wh, wl in ((( qth, qtl), wqh, wql),
                                             ((kth, ktl), wkh, wkl)):
                        for ib in range(4):
                            ps = psp.tile([128, 1024], F32, tag="ps")
                            xs = slice(ib * 512, (ib + 1) * 512)
                            for d in range(8):
                                nc.tensor.matmul(ps[:, 0:512], wh[:, d, :],
                                                 xhi[:, d, xs],
                                                 start=(d == 0), stop=False)
                                nc.tensor.matmul(ps[:, 0:512], wh[:, d, :],
                                                 xlo[:, d, xs],
                                                 start=False, stop=False)
                                nc.tensor.matmul(ps[:, 0:512], wl[:, d, :],
                                                 xhi[:, d, xs],
                                                 start=False, stop=(d == 7))
                            os_ = slice(ib * 512, (ib + 1) * 512)
                            nc.vector.tensor_copy(dh[:, os_], ps[:, 0:512])
                            nc.vector.tensor_sub(dl[:, os_], ps[:, 0:512],
                                                 dh[:, os_])

                    # -- attention --
                    av = avp.tile([128, N], F32, tag="av")
                    for c4 in range(4):   # zero-fill: set has_written everywhere
                        nc.tensor.matmul(av[:, c4 * 512:(c4 + 1) * 512],
                                         zl[:], zr[:], start=True, stop=False)
                    for jt in range(16):
                        js = slice(jt * 128, (jt + 1) * 128)
                        for hh in range(2):
                            base = hh * 64
                            bs = slice(base, base + 64)
                            h4 = 2 * pp + hh
                            tp = (base, 0)
                            accs = []
                            es = []
                            for ihalf in range(2):
                                s = psp.tile([128, 1024], F32, tag="ps")
                                for c2 in range(2):
                                    ss = slice(c2 * 512, (c2 + 1) * 512)
                                    i0 = ihalf * 1024 + c2 * 512
                                    qs = slice(i0, i0 + 512)
                                    nc.tensor.matmul(s[:, ss], kth[bs, js],
                                                     qth[bs, qs], start=True,
                                                     stop=False,
                                                     tile_position=tp)
                                    nc.tensor.matmul(s[:, ss], kth[bs, js],
                                                     qtl[bs, qs], start=False,
                                                     stop=False,
                                                     tile_position=tp)
                                    nc.tensor.matmul(s[:, ss], ktl[bs, js],
                                                     qth[bs, qs], start=False,
                                                     stop=True,
                                                     tile_position=tp)
                                e = epool.tile([128, 1024], F16, tag="e")
                                a = small.tile([128, 1], F32, tag=f"acc{ihalf}")
                                nc.scalar.activation(e[:], s[:], EXP,
                                                     bias=ebias[:],
                                                     accum_out=a[:])
                                es.append(e)
                                accs.append(a)
                            cs = small.tile([128, 1], F32, tag="c")
                            nc.vector.tensor_add(cs[:], accs[0][:], accs[1][:])
                            rc = small.tile([128, 1], F32, tag="rc")
                            nc.vector.reciprocal(rc[:], cs[:])
                            nc.vector.tensor_scalar_mul(
                                vt2[hh][:, base:base + 64],
                                v4[:, jt, h4 * 64:(h4 + 1) * 64], rc[:])
                            for c4 in range(4):
                                nc.tensor.matmul(
                                    av[:, c4 * 512:(c4 + 1) * 512],
                                    vt2[hh][:],
                                    es[c4 // 2][:, (c4 % 2) * 512:(c4 % 2) * 512 + 512],
                                    start=False, stop=(jt == 15))

                    # -- O projection --
                    avh = avtpool.tile([128, N], F16, tag="avh")
                    avl = avtpool.tile([128, N], F16, tag="avl")
                    nc.vector.tensor_copy(avh[:], av[:])
                    nc.vector.tensor_sub(avl[:], av[:], avh[:])
                    for hh in range(2):
                        base = hh * 64
                        a2ps = psp.tile([128, 1024], F32, tag="ps")
                        for c2 in range(2):   # zero-fill the two banks
                            nc.tensor.matmul(a2ps[:, c2 * 512:(c2 + 1) * 512],
                                             zl[:], zr[:], start=True, stop=False)
                        for si, src in enumerate((avh, avl)):
                            av_v = src[base:base + 64, :].rearrange(
                                "p (r m) -> p m r", m=16)
                            for par in range(2):
                                for c2 in range(2):
                                    m0 = 8 * c2 + par
                                    nc.tensor.matmul(
                                        a2ps[:, c2 * 512:(c2 + 1) * 512],
                                        idr[base:base + 64,
                                            128 * par:128 * par + 128],
                                        av_v[:, m0:m0 + 7:2, :],
                                        start=False,
                                        stop=(si == 1 and par == 1),
                                        tile_position=(base, 0))
                        a2h = a2pool.tile([128, 1024], F16, tag="a2h")
                        a2l = a2pool.tile([128, 1024], F16, tag="a2l")
                        nc.vector.tensor_copy(a2h[:], a2ps[:])
                        nc.vector.tensor_sub(a2l[:], a2ps[:], a2h[:])
                        pso = psp.tile([128, 1024], F32, tag="ps")
                        for dblk in range(2):
                            ds_ = slice(dblk * 512, (dblk + 1) * 512)
                            for k in range(8):
                                ks = slice(k * 128, (k + 1) * 128)
                                nc.tensor.matmul(pso[:, ds_], a2h[:, ks],
                                                 wohi[:, k, ds_],
                                                 start=(k == 0), stop=False)
                                nc.tensor.matmul(pso[:, ds_], a2l[:, ks],
                                                 wohi[:, k, ds_],
                                                 start=False, stop=False)
                                nc.tensor.matmul(pso[:, ds_], a2h[:, ks],
                                                 wolo[:, k, ds_],
                                                 start=False, stop=(k == 7))
                        o_sb = opool.tile([128, D], F32, tag="o")
                        nc.vector.tensor_copy(o_sb[:], pso[:])
                        g = 2 * pair + hh
                        nc.sync.dma_start(out_d[g * 128:(g + 1) * 128, :], o_sb[:])

    nc.compile()
    return nc


def _get_compiled():
    global _compiled
    if _compiled is None:
        _compiled = _build()
    return _compiled


def _run(in_maps, **kwargs):
    nc = _get_compiled()
    return run_bass_kernel_spmd(nc, in_maps, core_ids=list(range(NCORES)),
                                **kwargs)


def _make_in_maps(inputs):
    X = np.ascontiguousarray(np.asarray(inputs["X"], dtype=np.float32))
    WQ = np.ascontiguousarray(np.asarray(inputs["W_Q"], dtype=np.float32))
    WK = np.ascontiguousarray(np.asarray(inputs["W_K"], dtype=np.float32))
    WV = np.ascontiguousarray(np.asarray(inputs["W_V"], dtype=np.float32))
    WO = np.ascontiguousarray(np.asarray(inputs["W_O"], dtype=np.float32))
    ident = _identity_np()
    in_maps = []
    for c in range(NCORES):
        b = c // 2
        hs = (c % 2) * HPC
        in_maps.append({
            "X": X[b],
            "WQ": np.ascontiguousarray(WQ[hs:hs + HPC]),
            "WK": np.ascontiguousarray(WK[hs:hs + HPC]),
            "WV": np.ascontiguousarray(WV[hs:hs + HPC]),
            "WO": WO,
            "ID": ident,
        })
    return in_maps


def _assemble(results):
    out = np.empty((B, N, D), np.float32)
    for c in range(NCORES):
        b = c // 2
        r0 = (c % 2) * HPC * 128
        out[b, r0:r0 + HPC * 128, :] = results[c]["OUT"]
    return out


def kernel(**inputs) -> np.ndarray:
    res = _run(_make_in_maps(inputs))
    return _assemble(res.results)


def kernel_profiled(inputs):
    """Returns (output, BassKernelResults-with-trace) for test harnesses."""
    res = _run(_make_in_maps(inputs), trace=True)
    return _assemble(res.results), res


# revision 12
# speedup vs baseline: 1.0270x; 1.0270x over previous
"""Trainium2 Bass kernel for CausalSelfAttention variant (B=4, N=2048, D=1024, H=16, dk=dv=64).

Reference quirks faithfully implemented:
  - softmax over axis=2 (query axis): A[:, j] normalized over i (column softmax)
  - no 1/sqrt(dk) scaling
  - raw reshape (B,H,N,dv) -> (B,N,H*dv): output rows g*128:(g+1)*128 of batch b
    depend only on head g: out[b, g*128+r, :] = AV[b,g].reshape(128,1024)[r] @ W_O

Sharding (8 cores): core c handles batch b=c//2, global heads (c%2)*8 .. +8.
Each core produces out[b, (c%2)*1024 : +1024, :].

Precision scheme (target ~3e-4 vs fp32 reference): every fp32 value feeding the
softmax exponent goes through an fp16 hi+lo split (hi = fp16(x), lo = fp16(x-hi))
and products are computed as hi*hi + hi*lo + lo*hi (3-pass, error ~2^-21).
That covers X^T, W_Q/W_K, the Q/K projections and S^T itself. exp uses a
constant -14 bias so E=exp(S-14) and Vtil=V*(exp(14)/c) both fit fp16; the AV
matmul then runs single-pass fp16 (rounding averages out across the softmax
support). The O projection re-accumulates the exact AV rearrangement in PSUM
and runs 3-pass fp16 against split W_O.
"""

import numpy as np
from contextlib import ExitStack

import concourse.bass as bass
import concourse.tile as tile
from concourse import bacc, mybir
from concourse.bass_utils import run_bass_kernel_spmd

B, N, D, H, DK, DV = 4, 2048, 1024, 16, 64, 64
NCORES = 8
HPC = 8          # heads per core
F32 = mybir.dt.float32
F16 = mybir.dt.float16
EXP = mybir.ActivationFunctionType.Exp
EXP_BIAS = -14.0

_compiled = None


def _identity_np():
    # cols 0:256 : ID[p, 192*par + p%64] = 1  (avt -> avt2 rearrangement lhsT)
    # cols 256:384: plain I128 (PE-transpose identity)
    ident = np.zeros((128, 384), np.float32)
    for p in range(128):
        for par in range(2):
            ident[p, 192 * par + (p % 64)] = 1.0
        ident[p, 256 + p] = 1.0
    return ident


def _build():
    nc = bacc.Bacc("TRN2", target_bir_lowering=False, debug=False,
                   num_devices=NCORES)
    x_d = nc.dram_tensor("X", [N, D], F32, kind="ExternalInput").ap()
    wq_d = nc.dram_tensor("WQ", [HPC, D, DK], F32, kind="ExternalInput").ap()
    wk_d = nc.dram_tensor("WK", [HPC, D, DK], F32, kind="ExternalInput").ap()
    wv_d = nc.dram_tensor("WV", [HPC, D, DV], F32, kind="ExternalInput").ap()
    wo_d = nc.dram_tensor("WO", [D, D], F32, kind="ExternalInput").ap()
    id_d = nc.dram_tensor("ID", [128, 384], F32, kind="ExternalInput").ap()
    out_d = nc.dram_tensor("OUT", [HPC * 128, D], F32, kind="ExternalOutput").ap()

    with tile.TileContext(nc) as tc:
        with ExitStack() as ctx:
            persist = ctx.enter_context(tc.tile_pool(name="persist", bufs=1))
            stage = ctx.enter_context(tc.tile_pool(name="stage", bufs=2))
            wpool = ctx.enter_context(tc.tile_pool(name="w", bufs=1))
            vpool = ctx.enter_context(tc.tile_pool(name="v", bufs=1))
            qkpool = ctx.enter_context(tc.tile_pool(name="qk", bufs=2))
            sxpool = ctx.enter_context(tc.tile_pool(name="sx", bufs=1))
            epool = ctx.enter_context(tc.tile_pool(name="e", bufs=3))
            avtpool = ctx.enter_context(tc.tile_pool(name="avt", bufs=1))
            a2pool = ctx.enter_context(tc.tile_pool(name="a2", bufs=2))
            opool = ctx.enter_context(tc.tile_pool(name="o", bufs=2))
            small = ctx.enter_context(tc.tile_pool(name="sm", bufs=8))
            zpool = ctx.enter_context(tc.tile_pool(name="z", bufs=1))
            psp = ctx.enter_context(tc.tile_pool(name="ps", bufs=2, space="PSUM"))
            avp = ctx.enter_context(tc.tile_pool(name="avp", bufs=1, space="PSUM"))

            # ---- one-time init ----
            xhi = persist.tile([128, 8, N], F16, tag="xhi")      # X^T hi [d, i]
            xlo = persist.tile([128, 8, N], F16, tag="xlo")      # X^T lo
            wohi = persist.tile([128, 8, D], F16, tag="wohi")    # WO[128k+p, :]
            wolo = persist.tile([128, 8, D], F16, tag="wolo")
            idr = persist.tile([128, 384], F16, tag="idr")

            st = stage.tile([128, 1024], F32, tag="stg")
            nc.sync.dma_start(st[:, 0:384], id_d[:])
            nc.vector.tensor_copy(idr[:], st[:, 0:384])
            idT = idr[:, 256:384]

            for k in range(8):
                st = stage.tile([128, 1024], F32, tag="stg")
                nc.sync.dma_start(st[:], wo_d[k * 128:(k + 1) * 128, :])
                nc.vector.tensor_copy(wohi[:, k, :], st[:])
                nc.vector.tensor_sub(wolo[:, k, :], st[:], wohi[:, k, :])

            # X -> fp16 hi/lo -> PE-transpose into xhi/xlo
            for it in range(16):
                st = stage.tile([128, 1024], F32, tag="stg")
                nc.sync.dma_start(st[:], x_d[it * 128:(it + 1) * 128, :])
                sthi = stage.tile([128, D], F16, tag="sthi")
                stlo = stage.tile([128, D], F16, tag="stlo")
                nc.vector.tensor_copy(sthi[:], st[:])
                nc.vector.tensor_sub(stlo[:], st[:], sthi[:])
                for td in range(8):
                    ph = psp.tile([128, 128], F16, tag="ps")
                    nc.tensor.transpose(ph[:], sthi[:, td * 128:(td + 1) * 128],
                                        idT)
                    nc.vector.tensor_copy(
                        xhi[:, td, it * 128:(it + 1) * 128], ph[:])
                    pl = psp.tile([128, 128], F16, tag="ps")
                    nc.tensor.transpose(pl[:], stlo[:, td * 128:(td + 1) * 128],
                                        idT)
                    nc.scalar.copy(
                        xlo[:, td, it * 128:(it + 1) * 128], pl[:])

            z32 = zpool.tile([128, 128], F32, tag="z32")
            nc.vector.memset(z32[:], 0.0)
            z512 = zpool.tile([1, 512], F32, tag="z512")
            nc.vector.memset(z512[:], 0.0)
            ebias = zpool.tile([128, 1], F32, tag="ebias")
            nc.vector.memset(ebias[:], EXP_BIAS)
            zl = zpool.tile([1, 128], F16, tag="zl")
            zr = zpool.tile([1, 512], F16, tag="zr")
            nc.vector.tensor_copy(zl[:], z32[0:1, :])
            nc.vector.tensor_copy(zr[:], z512[:])
            # persistent [128, 128] Vtil tiles; the unused half stays zero so a
            # full-M matmul adds zero rows to the other head's AV partitions
            vt2 = []
            for hh in range(2):
                t = zpool.tile([128, 128], F16, tag=f"vt2_{hh}")
                nc.vector.tensor_copy(t[:], z32[:])
                vt2.append(t)

            for p2 in range(2):             # groups of 4 heads
                # V projection for heads 4*p2 .. 4*p2+3, packed N=256
                wv4 = wpool.tile([128, 8, 256], F16, tag="wv4")
                for h4 in range(4):
                    g = 4 * p2 + h4
                    st = stage.tile([128, 1024], F32, tag="stg")
                    stv = st[:, 0:512].rearrange("p (t k) -> p t k", k=64)
                    nc.sync.dma_start(
                        stv, wv_d[g].rearrange("(t p) k -> p t k", p=128))
                    nc.vector.tensor_copy(wv4[:, :, h4 * 64:(h4 + 1) * 64], stv)
                v4 = vpool.tile([128, 16, 256], F16, tag="v4")
                for jt in range(16):
                    psv = psp.tile([128, 1024], F32, tag="ps")
                    for d in range(8):
                        nc.tensor.matmul(psv[:, 0:256],
                                         xhi[:, d, jt * 128:(jt + 1) * 128],
                                         wv4[:, d, :],
                                         start=(d == 0), stop=(d == 7))
                    nc.vector.tensor_copy(v4[:, jt, :], psv[:, 0:256])

                for pp in range(2):         # head pairs within the group
                    pair = 2 * p2 + pp
                    # -- load packed W for the pair, split hi/lo --
                    wqh = wpool.tile([128, 8, 128], F16, tag="wqh")
                    wql = wpool.tile([128, 8, 128], F16, tag="wql")
                    wkh = wpool.tile([128, 8, 128], F16, tag="wkh")
                    wkl = wpool.tile([128, 8, 128], F16, tag="wkl")
                    for (wh, wl), wsrc in (((wqh, wql), wq_d),
                                           ((wkh, wkl), wk_d)):
                        st = stage.tile([128, 1024], F32, tag="stg")
                        stw = st[:].rearrange("p (t k) -> p t k", k=128)
                        for hh in range(2):
                            g = 2 * pair + hh
                            nc.sync.dma_start(
                                stw[:, :, hh * 64:(hh + 1) * 64],
                                wsrc[g].rearrange("(t p) k -> p t k", p=128))
                        nc.vector.tensor_copy(wh[:], stw)
                        nc.vector.tensor_sub(wl[:].rearrange("p t k -> p (t k)"),
                                             st[:],
                                             wh[:].rearrange("p t k -> p (t k)"))

                    # -- QK projections (3-pass split) --
                    # pair-packed hi tiles for the hi*hi S pass, plus per-head
                    # stacked tiles kxs=[Khi;Klo], qxs=[Qlo;Qhi] so both S
                    # correction terms run as ONE K=128 matmul.
                    qth = qkpool.tile([128, N], F16, tag="qth")
                    kth = qkpool.tile([128, N], F16, tag="kth")
                    qxs = [sxpool.tile([128, N], F16, name=f"qxs{h}",
                                       tag=f"qxs{h}") for h in range(2)]
                    kxs = [sxpool.tile([128, N], F16, name=f"kxs{h}",
                                       tag=f"kxs{h}") for h in range(2)]
                    for hi_t, (loA, hiA, hiB, loB), wh, wl in (
                            (qth, (qxs[0][0:64, :], qxs[0][64:128, :],
                                   qxs[1][64:128, :], qxs[1][0:64, :]),
                             wqh, wql),
                            (kth, (kxs[0][64:128, :], kxs[0][0:64, :],
                                   kxs[1][0:64, :], kxs[1][64:128, :]),
                             wkh, wkl)):
                        # loA/loB: where head A's / B's lo part belongs;
                        # hiA/hiB: where the hi parts belong in the stacked tile
                        for ib in range(4):
                            ps = psp.tile([128, 1024], F32, tag="ps")
                            xs = slice(ib * 512, (ib + 1) * 512)
                            for d in range(8):
                                nc.tensor.matmul(ps[:, 0:512], wh[:, d, :],
                                                 xhi[:, d, xs],
                                                 start=(d == 0), stop=False)
                                nc.tensor.matmul(ps[:, 0:512], wh[:, d, :],
                                                 xlo[:, d, xs],
                                                 start=False, stop=False)
                                nc.tensor.matmul(ps[:, 0:512], wl[:, d, :],
                                                 xhi[:, d, xs],
                                                 start=False, stop=(d == 7))
                            os_ = slice(ib * 512, (ib + 1) * 512)
                            nc.vector.tensor_copy(hi_t[:, os_], ps[:, 0:512])
                            # head A lo: partitions 0:64 stay aligned
                            if loA.tensor is qxs[0].tensor:
                                nc.vector.tensor_sub(loA[:, os_], ps[0:64, 0:512],
                                                     hi_t[0:64, os_])
                            else:  # K-side: lo_A belongs at partitions 64:128
                                tl = stage.tile([128, 512], F16, tag="tlo")
                                nc.vector.tensor_sub(tl[0:64, :], ps[0:64, 0:512],
                                                     hi_t[0:64, os_])
                                nc.sync.dma_start(loA[:, os_], tl[0:64, :])
                            # head B lo
                            if loB.tensor is kxs[1].tensor:
                                nc.vector.tensor_sub(loB[:, os_],
                                                     ps[64:128, 0:512],
                                                     hi_t[64:128, os_])
                            else:  # Q-side: lo_B must shift 64:128 -> 0:64
                                tl = stage.tile([128, 512], F16, tag="tlo")
                                nc.vector.tensor_sub(tl[64:128, :],
                                                     ps[64:128, 0:512],
                                                     hi_t[64:128, os_])
                                nc.sync.dma_start(loB[:, os_], tl[64:128, :])
                            # hi parts into the stacked tiles
                            if hiA.base_partition() == 0:
                                nc.vector.tensor_copy(hiA[:, os_],
                                                      hi_t[0:64, os_])
                            else:
                                nc.sync.dma_start(hiA[:, os_], hi_t[0:64, os_])
                            if hiB.base_partition() == 64:
                                nc.vector.tensor_copy(hiB[:, os_],
                                                      hi_t[64:128, os_])
                            else:
                                nc.sync.dma_start(hiB[:, os_], hi_t[64:128, os_])

                    # -- attention --
                    av = avp.tile([128, N], F32, tag="av")
                    for c4 in range(4):   # zero-fill: set has_written everywhere
                        nc.tensor.matmul(av[:, c4 * 512:(c4 + 1) * 512],
                                         zl[:], zr[:], start=True, stop=False)
                    for jt in range(16):
                        js = slice(jt * 128, (jt + 1) * 128)
                        for hh in range(2):
                            base = hh * 64
                            bs = slice(base, base + 64)
                            h4 = 2 * pp + hh
                            tp = (base, 0)
                            accs = []
                            es = []
                            for ihalf in range(2):
                                s = psp.tile([128, 1024], F32, tag="ps")
                                for c2 in range(2):
                                    ss = slice(c2 * 512, (c2 + 1) * 512)
                                    i0 = ihalf * 1024 + c2 * 512
                                    qs = slice(i0, i0 + 512)
                                    nc.tensor.matmul(s[:, ss], kth[bs, js],
                                                     qth[bs, qs], start=True,
                                                     stop=False,
                                                     tile_position=tp)
                                    nc.tensor.matmul(s[:, ss], kxs[hh][:, js],
                                                     qxs[hh][:, qs],
                                                     start=False, stop=True)
                                e = epool.tile([128, 1024], F16, tag="e")
                                a = small.tile([128, 1], F32, tag=f"acc{ihalf}")
                                nc.scalar.activation(e[:], s[:], EXP,
                                                     bias=ebias[:],
                                                     accum_out=a[:])
                                es.append(e)
                                accs.append(a)
                            cs = small.tile([128, 1], F32, tag="c")
                            nc.vector.tensor_add(cs[:], accs[0][:], accs[1][:])
                            rc = small.tile([128, 1], F32, tag="rc")
                            nc.vector.reciprocal(rc[:], cs[:])
                            nc.vector.tensor_scalar_mul(
                                vt2[hh][:, base:base + 64],
                                v4[:, jt, h4 * 64:(h4 + 1) * 64], rc[:])
                            for c4 in range(4):
                                nc.tensor.matmul(
                                    av[:, c4 * 512:(c4 + 1) * 512],
                                    vt2[hh][:],
                                    es[c4 // 2][:, (c4 % 2) * 512:(c4 % 2) * 512 + 512],
                                    start=False, stop=(jt == 15))

                    # -- O projection --
                    avh = avtpool.tile([128, N], F16, tag="avh")
                    avl = avtpool.tile([128, N], F16, tag="avl")
                    nc.vector.tensor_copy(avh[:], av[:])
                    nc.vector.tensor_sub(avl[:], av[:], avh[:])
                    for hh in range(2):
                        base = hh * 64
                        a2ps = psp.tile([128, 1024], F32, tag="ps")
                        for c2 in range(2):   # zero-fill the two banks
                            nc.tensor.matmul(a2ps[:, c2 * 512:(c2 + 1) * 512],
                                             zl[:], zr[:], start=True, stop=False)
                        for si, src in enumerate((avh, avl)):
                            av_v = src[base:base + 64, :].rearrange(
                                "p (r m) -> p m r", m=16)
                            for par in range(2):
                                for c2 in range(2):
                                    m0 = 8 * c2 + par
                                    nc.tensor.matmul(
                                        a2ps[:, c2 * 512:(c2 + 1) * 512],
                                        idr[base:base + 64,
                                            128 * par:128 * par + 128],
                                        av_v[:, m0:m0 + 7:2, :],
                                        start=False,
                                        stop=(si == 1 and par == 1),
                                        tile_position=(base, 0))
                        a2h = a2pool.tile([128, 1024], F16, tag="a2h")
                        a2l = a2pool.tile([128, 1024], F16, tag="a2l")
                        nc.vector.tensor_copy(a2h[:], a2ps[:])
                        nc.vector.tensor_sub(a2l[:], a2ps[:], a2h[:])
                        pso = psp.tile([128, 1024], F32, tag="ps")
                        for dblk in range(2):
                            ds_ = slice(dblk * 512, (dblk + 1) * 512)
                            for k in range(8):
                                ks = slice(k * 128, (k + 1) * 128)
                                nc.tensor.matmul(pso[:, ds_], a2h[:, ks],
                                                 wohi[:, k, ds_],
                                                 start=(k == 0), stop=False)
                                nc.tensor.matmul(pso[:, ds_], a2l[:, ks],
                                                 wohi[:, k, ds_],
                                                 start=False, stop=False)
                                nc.tensor.matmul(pso[:, ds_], a2h[:, ks],
                                                 wolo[:, k, ds_],
                                                 start=False, stop=(k == 7))
                        o_sb = opool.tile([128, D], F32, tag="o")
                        nc.vector.tensor_copy(o_sb[:], pso[:])
                        g = 2 * pair + hh
                        nc.sync.dma_start(out_d[g * 128:(g + 1) * 128, :], o_sb[:])

    nc.compile()
    return nc


def _get_compiled():
    global _compiled
    if _compiled is None:
        _compiled = _build()
    return _compiled


def _run(in_maps, **kwargs):
    nc = _get_compiled()
    return run_bass_kernel_spmd(nc, in_maps, core_ids=list(range(NCORES)),
                                **kwargs)


def _make_in_maps(inputs):
    X = np.ascontiguousarray(np.asarray(inputs["X"], dtype=np.float32))
    WQ = np.ascontiguousarray(np.asarray(inputs["W_Q"], dtype=np.float32))
    WK = np.ascontiguousarray(np.asarray(inputs["W_K"], dtype=np.float32))
    WV = np.ascontiguousarray(np.asarray(inputs["W_V"], dtype=np.float32))
    WO = np.ascontiguousarray(np.asarray(inputs["W_O"], dtype=np.float32))
    ident = _identity_np()
    in_maps = []
    for c in range(NCORES):
        b = c // 2
        hs = (c % 2) * HPC
        in_maps.append({
            "X": X[b],
            "WQ": np.ascontiguousarray(WQ[hs:hs + HPC]),
            "WK": np.ascontiguousarray(WK[hs:hs + HPC]),
            "WV": np.ascontiguousarray(WV[hs:hs + HPC]),
            "WO": WO,
            "ID": ident,
        })
    return in_maps


def _assemble(results):
    out = np.empty((B, N, D), np.float32)
    for c in range(NCORES):
        b = c // 2
        r0 = (c % 2) * HPC * 128
        out[b, r0:r0 + HPC * 128, :] = results[c]["OUT"]
    return out


def kernel(**inputs) -> np.ndarray:
    res = _run(_make_in_maps(inputs))
    return _assemble(res.results)


def kernel_profiled(inputs):
    """Returns (output, BassKernelResults-with-trace) for test harnesses."""
    res = _run(_make_in_maps(inputs), trace=True)
    return _assemble(res.results), res
